# revision 92
# baseline (speedup 1.0000x reference)
"""MoE logistic regression kernel for 8 Trainium2 NeuronCores.

Math (after dead-code elimination of the reference's unused router path):
    noise_logits = x @ noise_w.T + noise_b            # [B, E]
    top8 = top_k(noise_logits, 8)
    gates = softmax over the top-8 entries (others 0)
    expert = sigmoid(x @ expert_w.T + expert_b)       # [B, E]
    out[b] = sum_e gates[b,e] * expert[b,e]           # [B, 1]

Sharding: batch split 8 ways (2048 rows/core); weights replicated.

Implementation notes:
- x streams in fp8 e4m3 and every matmul runs in DoubleRow perf mode
  (2 contraction rows per PE pass), so the PE is ~4x faster than the
  DMA pipe and the kernel is bound by the serial DMA transfer pipe:
  ~2us start + x 25.3us + weights 3.2us. e4m3's 3 mantissa bits would
  be too lossy for the weights, so each weight chunk is sent as an
  e4m3 hi + e4m3 residual-lo PAIR (both x512 host-prescaled; the hi+lo
  matmuls accumulate into the same PSUM group, and the single 1/512
  descale folds into the ACT activation scale). Weight error vanishes;
  the remaining x-quantization error gives l2 rel err ~1.6e-2 vs the
  2e-2 gate (top-8 selection flips dominate). The extra lo matmuls are
  free: PE has 2x slack.
- DoubleRow operand layout per the executor: lhsT [128, 2, M] and
  rhs [128, 2, N] with plane i = contraction chunk 2k+i -- exactly a
  [:, 2g:2g+2, :] slice of the existing [128, k, *] layouts.
- Batch-tile-major stream; each tile's epilogue overlaps the next
  tile's transfers. Every DMA costs ~650ns of queue issue + HWDGE
  generation, hence >=8-chunk groups.
- PE p-state warm-up: dummy matmuls (junk SBUF, Pool-memset at ~60ns)
  burn the 0.65/1.2/2.4GHz ramp.
- The expert half of the weights/bias is HOST-NEGATED, so ONE ACT exp
  per tile produces e[0:64]=exp(noise_logit+nb) and
  e[64:128]=exp(-(expert_logit+eb)) straight off PSUM, and ONE
  [128,128] fp32 PE transpose per 128-col block lands both halves
  batch-major (PE slack makes fp32 transposes free; staying fp32
  avoids fp16 top-8 boundary collisions).
- DVE gating chain per tile: den=1+eB, sig=recip, Max8 per 128-block,
  zsum = one tensor_reduce over the Max8 outputs, es = e*sig in one
  strided mul, s4 via accumulating scalar_tensor_tensor selects, final
  s4 * recip(zsum). All ACT ops stay in the exp_and_others table set;
  ps stays DVE-only-read so its PSUM buffer frees when the chain
  drains. Tiles 4/5 stage in long-retired acc banks to dodge the
  2-deep pstr rotation.
- Per-tile outputs DMA from the [128, njs] result on the gpsimd SWDGE
  queue; the last tile's rides SP, idle by then.
"""

import sys

import numpy as np

if "/opt/trn_rl_repo" not in sys.path:
    sys.path.insert(0, "/opt/trn_rl_repo")

B, D, E, TOPK, NCORES = 16384, 4096, 64, 8, 8
BC = B // NCORES      # batch rows per core
NK = D // 128         # contraction chunks
WSCALE = 512.0        # host weight prescale (descaled in the epilogue)
# Tiles big-to-small: the per-tile epilogue chains must hide inside the
# following tiles' matmul windows; only the last (small) tile's chain is
# exposed as tail latency.
TILES = [512, 512, 512, 256, 128, 128]          # batch tile widths
OFFS = [sum(TILES[:i]) for i in range(len(TILES))]
assert sum(TILES) == BC
# DMA grouping in k-chunks per tile (each DMA costs ~650ns of queue
# issue+generation, so groups stay >= 8 chunks); the final tile tapers
# so almost no matmul work remains after the last byte lands. All
# groups have even size and even alignment (DoubleRow consumes pairs).
GROUPS = [[8, 8, 8, 8]] * 5 + [[16, 8, 4, 2, 2]]

_cached = {}


def _build_program():
    import concourse.bass as bass
    import concourse.tile as tile
    from concourse import bacc, mybir
    from concourse.masks import make_identity

    f32 = mybir.dt.float32
    f8e4 = mybir.dt.float8e4
    act = mybir.ActivationFunctionType
    alu = mybir.AluOpType
    drow = mybir.MatmulPerfMode.DoubleRow

    nc = bacc.Bacc("TRN2", target_bir_lowering=False, debug=False)
    # x fp8 (e4m3), per-tile partition-major blocks concatenated flat:
    # tile t occupies [128, NK, bt] at element offset 128*NK*OFFS[t], so
    # every group DMA is one contiguous gsz*bt-byte run per partition.
    xt = nc.dram_tensor("xt", [NK * 128 * BC], f8e4, kind="ExternalInput").ap()
    # weights as e4m3 hi + e4m3 residual lo, [128, (hi|lo), k, 128] flat
    wt0 = nc.dram_tensor("wt0", [128, 2 * 8 * 128], f8e4,
                         kind="ExternalInput").ap()
    wt1 = nc.dram_tensor("wt1", [128, 2 * (NK - 8) * 128], f8e4,
                         kind="ExternalInput").ap()
    bb = nc.dram_tensor("bb", [128, 1], f32, kind="ExternalInput").ap()
    out = nc.dram_tensor("out", [BC, 1], f32, kind="ExternalOutput").ap()

    with tile.TileContext(nc) as tc:
        with (
            tc.tile_pool(name="consts", bufs=1) as consts,
            tc.tile_pool(name="xpool", bufs=6) as xpool,
            tc.tile_pool(name="eppool", bufs=4) as eppool,
            tc.tile_pool(name="small", bufs=4) as small,
            tc.tile_pool(name="tvp", bufs=8) as tvp,
            tc.tile_pool(name="psacc", bufs=1, space=bass.MemorySpace.PSUM) as psacc,
            tc.tile_pool(name="pstr", bufs=2, space=bass.MemorySpace.PSUM) as pstr,
        ):
            # ---- constants ----
            accs = [psacc.tile([128, 512], f32, tag=f"acc{t}", name=f"acc{t}")
                    for t in range(len(TILES))]
            # PE p-state warm-up: matmul speed ramps 0.65->1.2->2.4 GHz only
            # after ~3us of continuous PE execution. Burn the ramp on dummy
            # 64-col matmuls (junk scratch via a Pool memset at ~60ns, no
            # DMA deps) so every real matmul runs at full clock. The last
            # acc's first real matmul start=True resets the bank.
            junk = consts.tile([128, 64], f32)
            nc.gpsimd.memset(junk, 0.0)
            for wi in range(17):
                nc.tensor.matmul(accs[-1][0:64, 0:64], lhsT=junk, rhs=junk,
                                 start=True, stop=True)
            # w0 (hi+lo) rides the Pool SWDGE queue, overlapping the SP
            # x-stream pipe start; the SP queue carries ONLY x.
            w0_sb = consts.tile([128, 2, 8, 128], f8e4)
            nc.gpsimd.dma_start(
                out=w0_sb, in_=wt0.rearrange("p (h g m) -> p h g m",
                                             h=2, g=8))
            bb_sb = consts.tile([128, 1], f32)
            nc.gpsimd.dma_start(out=bb_sb, in_=bb)
            # w1 hi/lo in two pieces each on ACT so they slot between x
            # groups on the shared transfer pipe
            w1_sb = consts.tile([128, 2, NK - 8, 128], f8e4)
            w1r = wt1.rearrange("p (h g m) -> p h g m", h=2, g=NK - 8)
            nc.scalar.dma_start(out=w1_sb[:, :, 0:12, :],
                                in_=w1r[:, :, 0:12, :])
            nc.scalar.dma_start(out=w1_sb[:, :, 12:24, :],
                                in_=w1r[:, :, 12:24, :])
            ident32 = consts.tile([128, 128], f32)
            make_identity(nc, ident32)
            # warm the ACT exp_and_others table during the DMA phase; every
            # later ACT op (Identity/Copy/Exp) stays in this one set.
            warm = consts.tile([1, 1], f32)
            nc.vector.memset(warm, 0.0)
            nc.scalar.add(warm, warm, bb_sb[0:1, 0:1])
            nc.scalar.activation(warm, warm, func=act.Exp)
            # tiles 0-3 stage their results here; one deferred DMA ships
            # them after tile 3's epilogue so no mid-stream output transfer
            # steals a slot on the (now binding) DMA pipe
            final_sb = consts.tile([128, 14], f32)

            for t, bt in enumerate(TILES):
                njs = bt // 128
                off = OFFS[t]
                acc = accs[t][:, 0:bt]
                # ---- stream tile t's contraction, accumulate logits.T ----
                # acc[0:64,:] = WSCALE*noise logits.T, acc[64:128,:] =
                # -WSCALE*expert logits.T (both pre-bias). DoubleRow pairs:
                # one hi + one lo matmul per (2k, 2k+1) chunk pair, all
                # accumulating into one PSUM group.
                base = 128 * NK * off
                xtile = xt[base:base + 128 * NK * bt].rearrange(
                    "(p k b) -> p k b", p=128, k=NK)
                k0 = 0
                for gsz in GROUPS[t]:
                    xk = xpool.tile([128, gsz, bt], f8e4, tag=f"xk{bt}_{gsz}")
                    nc.sync.dma_start(out=xk, in_=xtile[:, k0:k0 + gsz, :])
                    for g in range(0, gsz, 2):
                        k = k0 + g
                        if k < 8:
                            wh = w0_sb[:, 0, k:k + 2, :]
                            wl = w0_sb[:, 1, k:k + 2, :]
                        else:
                            wh = w1_sb[:, 0, k - 8:k - 6, :]
                            wl = w1_sb[:, 1, k - 8:k - 6, :]
                        xp = xk[:, g:g + 2, :]
                        nc.tensor.matmul(acc, lhsT=wh, rhs=xp,
                                         start=(k == 0), stop=False,
                                         perf_mode=drow)
                        nc.tensor.matmul(acc, lhsT=wl, rhs=xp,
                                         start=False, stop=(k == NK - 2),
                                         perf_mode=drow)
                    k0 += gsz

                # ---- epilogue for tile t (overlaps tile t+1's stream) ----
                # ONE exp for both halves straight off PSUM: the expert
                # weights/bias are host-negated, so e[0:64] = exp(nz+nb) and
                # e[64:128] = exp(-(ez+eb)) share the same +1/WSCALE scale.
                ecomb = eppool.tile([128, bt], f32, tag=f"ec{bt}")
                nc.scalar.activation(ecomb, accs[t][:, 0:bt], func=act.Exp,
                                     scale=1.0 / WSCALE, bias=bb_sb)
                # transpose to batch-major [128 batch, j, 0:64|64:128] in
                # one [128,128] transpose per j-block. Tiles 4/5 stage in
                # the long-retired acc0/acc1 banks so they never wait on
                # the 2-deep pstr rotation.
                if t < 4:
                    ps = pstr.tile([128, 4, 128], f32, tag="ps",
                                   name=f"ps{t}")
                    psC = [ps[:, j, :] for j in range(njs)]
                    psA = [ps[:, j, 0:64] for j in range(njs)]
                    psAall = ps[:, 0:njs, 0:64]
                    psBall = ps[:, 0:njs, 64:128]
                else:
                    psC = [accs[t - 4][:, j * 128:(j + 1) * 128]
                           for j in range(njs)]
                    psA = [accs[t - 4][:, j * 128:j * 128 + 64]
                           for j in range(njs)]
                    psB = [accs[t - 4][:, j * 128 + 64:(j + 1) * 128]
                           for j in range(njs)]
                for j in range(njs):
                    nc.tensor.transpose(psC[j],
                                        ecomb[:, j * 128:(j + 1) * 128],
                                        ident32)
                # den = 1 + eB; sigmoid = 1/den -- emitted before tv/zred so
                # the sig chain (which es and the final select depend on)
                # clears DVE first
                den = small.tile([128, 4, 64], f32, tag="den")
                if t < 4:
                    nc.vector.tensor_scalar_add(den[:, 0:njs, :], psBall,
                                                1.0)
                else:
                    for j in range(njs):
                        nc.vector.tensor_scalar_add(den[:, j, :], psB[j],
                                                    1.0)
                sig = small.tile([128, 4, 64], f32, tag="sig")
                nc.vector.reciprocal(sig[:, 0:njs, :], den[:, 0:njs, :])
                # top-8 on exp(v) (monotone => same selection as on v);
                # zsum = sum of the top-8 values in ONE reduce over tv
                tv = tvp.tile([128, 32], f32, tag="tv", name=f"tv{t}")
                for j in range(njs):
                    nc.vector.max(tv[:, j * 8:(j + 1) * 8], psA[j])
                zsum = small.tile([128, 4], f32, tag="zsum")
                nc.vector.tensor_reduce(
                    zsum[:, 0:njs],
                    tv.rearrange("p (j k) -> p j k", k=8)[:, 0:njs, :],
                    axis=mybir.AxisListType.X, op=alu.add)
                # es = e * sigmoid in ONE strided op (per-j for the last
                # tile, whose staging lives in a plain acc-bank slice). All
                # ps reads stay on DVE so the PSUM staging buffer frees as
                # soon as the DVE chain drains.
                es = small.tile([128, 4, 64], f32, tag="es")
                if t < 4:
                    nc.vector.tensor_mul(es[:, 0:njs, :], psAall,
                                         sig[:, 0:njs, :])
                else:
                    for j in range(njs):
                        nc.vector.tensor_mul(es[:, j, :], psA[j],
                                             sig[:, j, :])
                # s4 = sum of top-8 e*sigmoid (accumulating select)
                s4 = small.tile([128, 4], f32, tag="s4")
                scr = small.tile([128, 4, 64], f32, tag="scr")
                for j in range(njs):
                    nc.vector.scalar_tensor_tensor(
                        out=scr[:, j, :], in0=psA[j],
                        scalar=tv[:, j * 8 + 7:j * 8 + 8], in1=es[:, j, :],
                        op0=alu.is_ge, op1=alu.mult,
                        accum_out=s4[:, j:j + 1])
                rz = small.tile([128, 4], f32, tag="rz")
                nc.vector.reciprocal(rz[:, 0:njs], zsum[:, 0:njs])
                if t <= 3:
                    c0 = off // 128
                    nc.vector.tensor_mul(final_sb[:, c0:c0 + njs],
                                         s4[:, 0:njs], rz[:, 0:njs])
                    if t == 3:
                        nc.gpsimd.dma_start(
                            out=out[0:1792, :].rearrange(
                                "(j p) o -> p (j o)", j=14, p=128),
                            in_=final_sb)
                else:
                    fin = small.tile([128, 4], f32, tag="fin")
                    nc.vector.tensor_mul(fin[:, 0:njs], s4[:, 0:njs],
                                         rz[:, 0:njs])
                    out_t = out[off:off + bt, :].rearrange(
                        "(j p) o -> p (j o)", j=njs, p=128)
                    eng = nc.sync if t == len(TILES) - 1 else nc.gpsimd
                    eng.dma_start(out=out_t, in_=fin[:, 0:njs])

    nc.compile()
    return nc


def get_program():
    if "prog" not in _cached:
        _cached["prog"] = _build_program()
    return _cached["prog"]


def make_in_maps(x, noise_w, noise_b, expert_w, expert_b):
    """Host-side sharding: per-core transposed fp8(e4m3) x + hi/lo weights.

    The expert half is NEGATED (weights and bias) so the kernel computes
    exp(+scale*acc + bias) for all 128 logit rows in one ACT op:
    rows 64:128 then hold exp(-(expert_logit + expert_b)) directly.
    Each weight chunk ships as e4m3 hi + e4m3 residual lo (same x512
    prescale), summed in PSUM by two DoubleRow matmuls.
    """
    import ml_dtypes
    e4 = ml_dtypes.float8_e4m3
    w_comb = np.concatenate([noise_w, -expert_w], axis=0).astype(np.float32)
    wt32 = np.ascontiguousarray(w_comb.T) * np.float32(WSCALE)   # [D, 128]
    # partition p holds [nk, 128] for contraction rows nk*128+p
    wt = np.ascontiguousarray(
        wt32.reshape(NK, 128, 128).transpose(1, 0, 2).reshape(128, NK, 128))
    wh = wt.astype(e4)
    wl = (wt - wh.astype(np.float32)).astype(e4)
    whl = np.stack([wh, wl], axis=1)            # [128, 2, NK, 128]
    wt0 = np.ascontiguousarray(whl[:, :, :8, :].reshape(128, -1))
    wt1 = np.ascontiguousarray(whl[:, :, 8:, :].reshape(128, -1))
    bb = np.concatenate([noise_b, -expert_b]).reshape(128, 1).astype(
        np.float32)
    in_maps = []
    for c in range(NCORES):
        xs = np.ascontiguousarray(x[c * BC:(c + 1) * BC, :].T).astype(e4)
        # per tile: [D, bt] -> [128, NK, bt], concatenated flat
        blocks = []
        for t, bt in enumerate(TILES):
            blk = xs[:, OFFS[t]:OFFS[t] + bt].reshape(NK, 128, bt)
            blocks.append(blk.transpose(1, 0, 2).reshape(-1))
        xr = np.ascontiguousarray(np.concatenate(blocks))
        in_maps.append({"xt": xr, "wt0": wt0, "wt1": wt1, "bb": bb})
    return in_maps


def kernel(x, noise, router_w, router_b, noise_w, noise_b, expert_w, expert_b,
           _trace=False):
    from concourse.bass_utils import run_bass_kernel_spmd

    x = np.asarray(x, dtype=np.float32)
    nc = get_program()
    in_maps = make_in_maps(x, np.asarray(noise_w), np.asarray(noise_b),
                           np.asarray(expert_w), np.asarray(expert_b))
    res = run_bass_kernel_spmd(nc, in_maps, core_ids=list(range(NCORES)),
                               trace=_trace)
    out = np.concatenate([r["out"] for r in res.results], axis=0)
    if _trace:
        kernel.last_results = res
    return out


# revision 97
# speedup vs baseline: 1.0230x; 1.0230x over previous
"""MoE logistic regression kernel for 8 Trainium2 NeuronCores.

Math (after dead-code elimination of the reference's unused router path):
    noise_logits = x @ noise_w.T + noise_b            # [B, E]
    top8 = top_k(noise_logits, 8)
    gates = softmax over the top-8 entries (others 0)
    expert = sigmoid(x @ expert_w.T + expert_b)       # [B, E]
    out[b] = sum_e gates[b,e] * expert[b,e]           # [B, 1]

Sharding: batch split 8 ways (2048 rows/core); weights replicated.

Implementation notes:
- x streams in fp8 e4m3 and every matmul runs in DoubleRow perf mode
  (2 contraction rows per PE pass), so the PE is ~4x faster than the
  DMA pipe and the kernel is bound by the serial DMA transfer pipe:
  ~2us start + x 25.3us + weights 3.2us. e4m3's 3 mantissa bits would
  be too lossy for the weights, so each weight chunk is sent as an
  e4m3 hi + e4m3 residual-lo PAIR (both x512 host-prescaled; the hi+lo
  matmuls accumulate into the same PSUM group, and the single 1/512
  descale folds into the ACT activation scale). Weight error vanishes;
  the remaining x-quantization error gives l2 rel err ~1.6e-2 vs the
  2e-2 gate (top-8 selection flips dominate). The extra lo matmuls are
  free: PE has 2x slack.
- DoubleRow operand layout per the executor: lhsT [128, 2, M] and
  rhs [128, 2, N] with plane i = contraction chunk 2k+i -- exactly a
  [:, 2g:2g+2, :] slice of the existing [128, k, *] layouts.
- Batch-tile-major stream; each tile's epilogue overlaps the next
  tile's transfers. Every DMA costs ~650ns of queue issue + HWDGE
  generation, hence >=8-chunk groups.
- PE p-state warm-up: dummy matmuls (junk SBUF, Pool-memset at ~60ns)
  burn the 0.65/1.2/2.4GHz ramp.
- The expert half of the weights/bias is HOST-NEGATED, so ONE ACT exp
  per tile produces e[0:64]=exp(noise_logit+nb) and
  e[64:128]=exp(-(expert_logit+eb)) straight off PSUM, and ONE
  [128,128] fp32 PE transpose per 128-col block lands both halves
  batch-major (PE slack makes fp32 transposes free; staying fp32
  avoids fp16 top-8 boundary collisions).
- DVE gating chain per tile: den=1+eB, sig=recip, Max8 per 128-block,
  zsum = one tensor_reduce over the Max8 outputs, es = e*sig in one
  strided mul, s4 via accumulating scalar_tensor_tensor selects, final
  s4 * recip(zsum). All ACT ops stay in the exp_and_others table set;
  ps stays DVE-only-read so its PSUM buffer frees when the chain
  drains. Tiles 4/5 stage in long-retired acc banks to dodge the
  2-deep pstr rotation.
- Per-tile outputs DMA from the [128, njs] result on the gpsimd SWDGE
  queue; the last tile's rides SP, idle by then.
"""

import sys

import numpy as np

if "/opt/trn_rl_repo" not in sys.path:
    sys.path.insert(0, "/opt/trn_rl_repo")

B, D, E, TOPK, NCORES = 16384, 4096, 64, 8, 8
BC = B // NCORES      # batch rows per core
NK = D // 128         # contraction chunks
WSCALE = 512.0        # host weight prescale (descaled in the epilogue)
# Tiles big-to-small: the per-tile epilogue chains must hide inside the
# following tiles' matmul windows; only the last (small) tile's chain is
# exposed as tail latency.
TILES = [512, 512, 512, 256, 128, 128]          # batch tile widths
OFFS = [sum(TILES[:i]) for i in range(len(TILES))]
assert sum(TILES) == BC
# DMA grouping in k-chunks per tile (each DMA costs ~650ns of queue
# issue+generation, so groups stay >= 8 chunks); the final tile tapers
# so almost no matmul work remains after the last byte lands. All
# groups have even size and even alignment (DoubleRow consumes pairs).
GROUPS = [[8, 8, 8, 8]] * 5 + [[16, 8, 4, 2, 2]]

_cached = {}


def _build_program():
    import concourse.bass as bass
    import concourse.tile as tile
    from concourse import bacc, mybir
    from concourse.masks import make_identity

    f32 = mybir.dt.float32
    f8e4 = mybir.dt.float8e4
    act = mybir.ActivationFunctionType
    alu = mybir.AluOpType
    drow = mybir.MatmulPerfMode.DoubleRow

    nc = bacc.Bacc("TRN2", target_bir_lowering=False, debug=False)
    # x fp8 (e4m3), per-tile partition-major blocks concatenated flat:
    # tile t occupies [128, NK, bt] at element offset 128*NK*OFFS[t], so
    # every group DMA is one contiguous gsz*bt-byte run per partition.
    xt = nc.dram_tensor("xt", [NK * 128 * BC], f8e4, kind="ExternalInput").ap()
    # weights as e4m3 hi [*, 128 outs] + e4m3 residual lo for the NOISE
    # half only [*, 64 outs]: expert-logit error only smooths through the
    # sigmoid (~0.2e-2 l2), so its lo correction isn't worth pipe bytes
    wt0 = nc.dram_tensor("wt0", [128, 8 * 128], f8e4,
                         kind="ExternalInput").ap()
    wt0l = nc.dram_tensor("wt0l", [128, 8 * 64], f8e4,
                          kind="ExternalInput").ap()
    wt1 = nc.dram_tensor("wt1", [128, (NK - 8) * 128], f8e4,
                         kind="ExternalInput").ap()
    wt1l = nc.dram_tensor("wt1l", [128, (NK - 8) * 64], f8e4,
                          kind="ExternalInput").ap()
    bb = nc.dram_tensor("bb", [128, 1], f32, kind="ExternalInput").ap()
    out = nc.dram_tensor("out", [BC, 1], f32, kind="ExternalOutput").ap()

    with tile.TileContext(nc) as tc:
        with (
            tc.tile_pool(name="consts", bufs=1) as consts,
            tc.tile_pool(name="xpool", bufs=6) as xpool,
            tc.tile_pool(name="eppool", bufs=4) as eppool,
            tc.tile_pool(name="small", bufs=4) as small,
            tc.tile_pool(name="tvp", bufs=8) as tvp,
            tc.tile_pool(name="psacc", bufs=1, space=bass.MemorySpace.PSUM) as psacc,
            tc.tile_pool(name="pstr", bufs=2, space=bass.MemorySpace.PSUM) as pstr,
        ):
            # ---- constants ----
            accs = [psacc.tile([128, 512], f32, tag=f"acc{t}", name=f"acc{t}")
                    for t in range(len(TILES))]
            # PE p-state warm-up: matmul speed ramps 0.65->1.2->2.4 GHz only
            # after ~3us of continuous PE execution. Burn the ramp on dummy
            # 64-col matmuls (junk scratch via a Pool memset at ~60ns, no
            # DMA deps) so every real matmul runs at full clock. The last
            # acc's first real matmul start=True resets the bank.
            junk = consts.tile([128, 64], f32)
            nc.gpsimd.memset(junk, 0.0)
            for wi in range(17):
                nc.tensor.matmul(accs[-1][0:64, 0:64], lhsT=junk, rhs=junk,
                                 start=True, stop=True)
            # w0 hi+lo ride the Pool SWDGE queue, overlapping the SP
            # x-stream pipe start; the SP queue carries ONLY x.
            w0_sb = consts.tile([128, 8, 128], f8e4)
            nc.gpsimd.dma_start(
                out=w0_sb, in_=wt0.rearrange("p (g m) -> p g m", g=8))
            w0l_sb = consts.tile([128, 8, 64], f8e4)
            nc.gpsimd.dma_start(
                out=w0l_sb, in_=wt0l.rearrange("p (g m) -> p g m", g=8))
            bb_sb = consts.tile([128, 1], f32)
            nc.gpsimd.dma_start(out=bb_sb, in_=bb)
            # w1 hi in two pieces + lo in one on ACT so they slot between
            # x groups on the shared transfer pipe
            w1_sb = consts.tile([128, NK - 8, 128], f8e4)
            w1r = wt1.rearrange("p (g m) -> p g m", g=NK - 8)
            nc.scalar.dma_start(out=w1_sb[:, 0:12, :], in_=w1r[:, 0:12, :])
            nc.scalar.dma_start(out=w1_sb[:, 12:24, :], in_=w1r[:, 12:24, :])
            w1l_sb = consts.tile([128, NK - 8, 64], f8e4)
            nc.scalar.dma_start(
                out=w1l_sb, in_=wt1l.rearrange("p (g m) -> p g m", g=NK - 8))
            ident32 = consts.tile([128, 128], f32)
            make_identity(nc, ident32)
            # warm the ACT exp_and_others table during the DMA phase; every
            # later ACT op (Identity/Copy/Exp) stays in this one set.
            warm = consts.tile([1, 1], f32)
            nc.vector.memset(warm, 0.0)
            nc.scalar.add(warm, warm, bb_sb[0:1, 0:1])
            nc.scalar.activation(warm, warm, func=act.Exp)
            # tiles 0-3 stage their results here; one deferred DMA ships
            # them after tile 3's epilogue so no mid-stream output transfer
            # steals a slot on the (now binding) DMA pipe
            final_sb = consts.tile([128, 14], f32)

            for t, bt in enumerate(TILES):
                njs = bt // 128
                off = OFFS[t]
                acc = accs[t][:, 0:bt]
                # ---- stream tile t's contraction, accumulate logits.T ----
                # acc[0:64,:] = WSCALE*noise logits.T, acc[64:128,:] =
                # -WSCALE*expert logits.T (both pre-bias). DoubleRow pairs:
                # one hi + one lo matmul per (2k, 2k+1) chunk pair, all
                # accumulating into one PSUM group.
                base = 128 * NK * off
                xtile = xt[base:base + 128 * NK * bt].rearrange(
                    "(p k b) -> p k b", p=128, k=NK)
                k0 = 0
                for gsz in GROUPS[t]:
                    xk = xpool.tile([128, gsz, bt], f8e4, tag=f"xk{bt}_{gsz}")
                    nc.sync.dma_start(out=xk, in_=xtile[:, k0:k0 + gsz, :])
                    for g in range(0, gsz, 2):
                        k = k0 + g
                        if k < 8:
                            wh = w0_sb[:, k:k + 2, :]
                            wl = w0l_sb[:, k:k + 2, :]
                        else:
                            wh = w1_sb[:, k - 8:k - 6, :]
                            wl = w1l_sb[:, k - 8:k - 6, :]
                        xp = xk[:, g:g + 2, :]
                        nc.tensor.matmul(acc, lhsT=wh, rhs=xp,
                                         start=(k == 0), stop=False,
                                         perf_mode=drow)
                        nc.tensor.matmul(accs[t][0:64, 0:bt], lhsT=wl,
                                         rhs=xp, start=False,
                                         stop=(k == NK - 2),
                                         perf_mode=drow,
                                         skip_group_check=True)
                    k0 += gsz

                # ---- epilogue for tile t (overlaps tile t+1's stream) ----
                # ONE exp for both halves straight off PSUM: the expert
                # weights/bias are host-negated, so e[0:64] = exp(nz+nb) and
                # e[64:128] = exp(-(ez+eb)) share the same +1/WSCALE scale.
                ecomb = eppool.tile([128, bt], f32, tag=f"ec{bt}")
                nc.scalar.activation(ecomb, accs[t][:, 0:bt], func=act.Exp,
                                     scale=1.0 / WSCALE, bias=bb_sb)
                # transpose to batch-major [128 batch, j, 0:64|64:128] in
                # one [128,128] transpose per j-block. Tiles 4/5 stage in
                # the long-retired acc0/acc1 banks so they never wait on
                # the 2-deep pstr rotation.
                if t < 4:
                    ps = pstr.tile([128, 4, 128], f32, tag="ps",
                                   name=f"ps{t}")
                    psC = [ps[:, j, :] for j in range(njs)]
                    psA = [ps[:, j, 0:64] for j in range(njs)]
                    psAall = ps[:, 0:njs, 0:64]
                    psBall = ps[:, 0:njs, 64:128]
                else:
                    psC = [accs[t - 4][:, j * 128:(j + 1) * 128]
                           for j in range(njs)]
                    psA = [accs[t - 4][:, j * 128:j * 128 + 64]
                           for j in range(njs)]
                    psB = [accs[t - 4][:, j * 128 + 64:(j + 1) * 128]
                           for j in range(njs)]
                for j in range(njs):
                    nc.tensor.transpose(psC[j],
                                        ecomb[:, j * 128:(j + 1) * 128],
                                        ident32)
                # den = 1 + eB; sigmoid = 1/den -- emitted before tv/zred so
                # the sig chain (which es and the final select depend on)
                # clears DVE first
                # den/es for the last two tiles run on the idle Pool engine
                # (their staging is in retired acc banks so Pool reading ps
                # cannot stall later transposes, and their outs ride SP so
                # no SWDGE gen competes on Pool); this keeps tile 5's DVE
                # chain from queueing behind tile 4's.
                den = small.tile([128, 4, 64], f32, tag="den")
                if t < 4:
                    nc.vector.tensor_scalar_add(den[:, 0:njs, :], psBall,
                                                1.0)
                else:
                    for j in range(njs):
                        nc.vector.tensor_scalar_add(den[:, j, :], psB[j],
                                                    1.0)
                sig = small.tile([128, 4, 64], f32, tag="sig")
                nc.vector.reciprocal(sig[:, 0:njs, :], den[:, 0:njs, :])
                # top-8 on exp(v) (monotone => same selection as on v);
                # zsum = sum of the top-8 values in ONE reduce over tv
                tv = tvp.tile([128, 32], f32, tag="tv", name=f"tv{t}")
                for j in range(njs):
                    nc.vector.max(tv[:, j * 8:(j + 1) * 8], psA[j])
                zsum = small.tile([128, 4], f32, tag="zsum")
                nc.vector.tensor_reduce(
                    zsum[:, 0:njs],
                    tv.rearrange("p (j k) -> p j k", k=8)[:, 0:njs, :],
                    axis=mybir.AxisListType.X, op=alu.add)
                # es = e * sigmoid in ONE strided op (per-j for the last
                # tile, whose staging lives in a plain acc-bank slice). All
                # ps reads stay on DVE so the PSUM staging buffer frees as
                # soon as the DVE chain drains.
                es = small.tile([128, 4, 64], f32, tag="es")
                if t < 4:
                    nc.vector.tensor_mul(es[:, 0:njs, :], psAall,
                                         sig[:, 0:njs, :])
                else:
                    for j in range(njs):
                        nc.vector.tensor_mul(es[:, j, :], psA[j],
                                             sig[:, j, :])
                # s4 = sum of top-8 e*sigmoid (accumulating select)
                s4 = small.tile([128, 4], f32, tag="s4")
                scr = small.tile([128, 4, 64], f32, tag="scr")
                for j in range(njs):
                    nc.vector.scalar_tensor_tensor(
                        out=scr[:, j, :], in0=psA[j],
                        scalar=tv[:, j * 8 + 7:j * 8 + 8], in1=es[:, j, :],
                        op0=alu.is_ge, op1=alu.mult,
                        accum_out=s4[:, j:j + 1])
                rz = small.tile([128, 4], f32, tag="rz")
                nc.vector.reciprocal(rz[:, 0:njs], zsum[:, 0:njs])
                if t <= 3:
                    c0 = off // 128
                    nc.vector.tensor_mul(final_sb[:, c0:c0 + njs],
                                         s4[:, 0:njs], rz[:, 0:njs])
                    if t == 3:
                        nc.gpsimd.dma_start(
                            out=out[0:1792, :].rearrange(
                                "(j p) o -> p (j o)", j=14, p=128),
                            in_=final_sb)
                else:
                    fin = small.tile([128, 4], f32, tag="fin")
                    nc.vector.tensor_mul(fin[:, 0:njs], s4[:, 0:njs],
                                         rz[:, 0:njs])
                    out_t = out[off:off + bt, :].rearrange(
                        "(j p) o -> p (j o)", j=njs, p=128)
                    nc.sync.dma_start(out=out_t, in_=fin[:, 0:njs])

    nc.compile()
    return nc


def get_program():
    if "prog" not in _cached:
        _cached["prog"] = _build_program()
    return _cached["prog"]


def make_in_maps(x, noise_w, noise_b, expert_w, expert_b):
    """Host-side sharding: per-core transposed fp8(e4m3) x + hi/lo weights.

    The expert half is NEGATED (weights and bias) so the kernel computes
    exp(+scale*acc + bias) for all 128 logit rows in one ACT op:
    rows 64:128 then hold exp(-(expert_logit + expert_b)) directly.
    Each weight chunk ships as e4m3 hi + e4m3 residual lo (same x512
    prescale), summed in PSUM by two DoubleRow matmuls.
    """
    import ml_dtypes
    e4 = ml_dtypes.float8_e4m3
    w_comb = np.concatenate([noise_w, -expert_w], axis=0).astype(np.float32)
    wt32 = np.ascontiguousarray(w_comb.T) * np.float32(WSCALE)   # [D, 128]
    # partition p holds [nk, 128] for contraction rows nk*128+p
    wt = np.ascontiguousarray(
        wt32.reshape(NK, 128, 128).transpose(1, 0, 2).reshape(128, NK, 128))
    wh = wt.astype(e4)
    wl = (wt - wh.astype(np.float32)).astype(e4)[:, :, 0:64]  # noise only
    wt0 = np.ascontiguousarray(wh[:, :8, :].reshape(128, -1))
    wt1 = np.ascontiguousarray(wh[:, 8:, :].reshape(128, -1))
    wt0l = np.ascontiguousarray(wl[:, :8, :].reshape(128, -1))
    wt1l = np.ascontiguousarray(wl[:, 8:, :].reshape(128, -1))
    bb = np.concatenate([noise_b, -expert_b]).reshape(128, 1).astype(
        np.float32)
    in_maps = []
    for c in range(NCORES):
        xs = np.ascontiguousarray(x[c * BC:(c + 1) * BC, :].T).astype(e4)
        # per tile: [D, bt] -> [128, NK, bt], concatenated flat
        blocks = []
        for t, bt in enumerate(TILES):
            blk = xs[:, OFFS[t]:OFFS[t] + bt].reshape(NK, 128, bt)
            blocks.append(blk.transpose(1, 0, 2).reshape(-1))
        xr = np.ascontiguousarray(np.concatenate(blocks))
        in_maps.append({"xt": xr, "wt0": wt0, "wt0l": wt0l, "wt1": wt1,
                        "wt1l": wt1l, "bb": bb})
    return in_maps


def kernel(x, noise, router_w, router_b, noise_w, noise_b, expert_w, expert_b,
           _trace=False):
    from concourse.bass_utils import run_bass_kernel_spmd

    x = np.asarray(x, dtype=np.float32)
    nc = get_program()
    in_maps = make_in_maps(x, np.asarray(noise_w), np.asarray(noise_b),
                           np.asarray(expert_w), np.asarray(expert_b))
    res = run_bass_kernel_spmd(nc, in_maps, core_ids=list(range(NCORES)),
                               trace=_trace)
    out = np.concatenate([r["out"] for r in res.results], axis=0)
    if _trace:
        kernel.last_results = res
    return out


# revision 101
# speedup vs baseline: 1.0396x; 1.0162x over previous
"""MoE logistic regression kernel for 8 Trainium2 NeuronCores.

Math (after dead-code elimination of the reference's unused router path):
    noise_logits = x @ noise_w.T + noise_b            # [B, E]
    top8 = top_k(noise_logits, 8)
    gates = softmax over the top-8 entries (others 0)
    expert = sigmoid(x @ expert_w.T + expert_b)       # [B, E]
    out[b] = sum_e gates[b,e] * expert[b,e]           # [B, 1]

Sharding: batch split 8 ways (2048 rows/core); weights replicated.

Implementation notes:
- x streams in fp8 e4m3 and every matmul runs in DoubleRow perf mode
  (2 contraction rows per PE pass), so the PE is ~4x faster than the
  DMA pipe and the kernel is bound by the serial DMA transfer pipe:
  ~2us start + x 25.3us + weights 2.4us. e4m3's 3 mantissa bits would
  be too lossy for the weights, so each chunk is sent as e4m3 hi plus
  an e4m3 residual-lo for the NOISE half only (the noise logits drive
  the top-8 selection; expert-logit error only smooths through the
  sigmoid). Both share the x512 host prescale, the hi+lo matmuls
  accumulate into one PSUM group (lo targets partitions 0:64,
  skip_group_check), and the single 1/512 descale folds into the ACT
  activation scale. l2 rel err ~1.61e-2 vs the 2e-2 gate (x-driven
  top-8 flips dominate). The extra lo matmuls are free: PE has slack.
- DoubleRow operand layout per the executor: lhsT [128, 2, M] and
  rhs [128, 2, N] with plane i = contraction chunk 2k+i -- exactly a
  [:, 2g:2g+2, :] slice of the existing [128, k, *] layouts.
- Batch-tile-major stream; each tile's epilogue overlaps the next
  tile's transfers. Every DMA costs ~650ns of queue issue + HWDGE
  generation, hence >=8-chunk groups.
- PE p-state warm-up: dummy matmuls (junk SBUF, Pool-memset at ~60ns)
  burn the 0.65/1.2/2.4GHz ramp.
- The expert half of the weights/bias is HOST-NEGATED, so ONE ACT exp
  per tile produces e[0:64]=exp(noise_logit+nb) and
  e[64:128]=exp(-(expert_logit+eb)) straight off PSUM, and ONE
  [128,128] fp32 PE transpose per 128-col block lands both halves
  batch-major (PE slack makes fp32 transposes free; staying fp32
  avoids fp16 top-8 boundary collisions).
- DVE gating chain per tile: den=1+eB, sig=recip, Max8 per 128-block,
  zsum = one tensor_reduce over the Max8 outputs, es = e*sig in one
  strided mul, s4 via accumulating scalar_tensor_tensor selects, final
  s4 * recip(zsum). All ACT ops stay in the exp_and_others table set;
  ps stays DVE-only-read so its PSUM buffer frees when the chain
  drains. Tiles 4/5 stage in long-retired acc banks to dodge the
  2-deep pstr rotation.
- Per-tile outputs DMA from the [128, njs] result on the gpsimd SWDGE
  queue; the last tile's rides SP, idle by then.
"""

import sys

import numpy as np

if "/opt/trn_rl_repo" not in sys.path:
    sys.path.insert(0, "/opt/trn_rl_repo")

B, D, E, TOPK, NCORES = 16384, 4096, 64, 8, 8
BC = B // NCORES      # batch rows per core
NK = D // 128         # contraction chunks
WSCALE = 512.0        # host weight prescale (descaled in the epilogue)
# Tiles big-to-small: the per-tile epilogue chains must hide inside the
# following tiles' matmul windows; only the last (small) tile's chain is
# exposed as tail latency.
TILES = [512, 512, 512, 256, 128, 128]          # batch tile widths
OFFS = [sum(TILES[:i]) for i in range(len(TILES))]
assert sum(TILES) == BC
# DMA grouping in k-chunks per tile (each DMA costs ~650ns of queue
# issue+generation, so groups stay >= 8 chunks); the final tile tapers
# so almost no matmul work remains after the last byte lands. All
# groups have even size and even alignment (DoubleRow consumes pairs).
GROUPS = [[8, 8, 8, 8]] * 5 + [[16, 8, 4, 2, 2]]

_cached = {}


def _build_program():
    import concourse.bass as bass
    import concourse.tile as tile
    from concourse import bacc, mybir
    from concourse.masks import make_identity

    f32 = mybir.dt.float32
    f8e4 = mybir.dt.float8e4
    act = mybir.ActivationFunctionType
    alu = mybir.AluOpType
    drow = mybir.MatmulPerfMode.DoubleRow

    nc = bacc.Bacc("TRN2", target_bir_lowering=False, debug=False)
    # x fp8 (e4m3), per-tile partition-major blocks concatenated flat:
    # tile t occupies [128, NK, bt] at element offset 128*NK*OFFS[t], so
    # every group DMA is one contiguous gsz*bt-byte run per partition.
    xt = nc.dram_tensor("xt", [NK * 128 * BC], f8e4, kind="ExternalInput").ap()
    # weights as e4m3 hi [*, 128 outs] + e4m3 residual lo for the NOISE
    # half only [*, 64 outs]: expert-logit error only smooths through the
    # sigmoid (~0.2e-2 l2), so its lo correction isn't worth pipe bytes
    wt0 = nc.dram_tensor("wt0", [128, 8 * 128], f8e4,
                         kind="ExternalInput").ap()
    wt0l = nc.dram_tensor("wt0l", [128, 8 * 64], f8e4,
                          kind="ExternalInput").ap()
    wt1 = nc.dram_tensor("wt1", [128, (NK - 8) * 128], f8e4,
                         kind="ExternalInput").ap()
    wt1l = nc.dram_tensor("wt1l", [128, (NK - 8) * 64], f8e4,
                          kind="ExternalInput").ap()
    bb = nc.dram_tensor("bb", [128, 1], f32, kind="ExternalInput").ap()
    out = nc.dram_tensor("out", [BC, 1], f32, kind="ExternalOutput").ap()

    with tile.TileContext(nc) as tc:
        with (
            tc.tile_pool(name="consts", bufs=1) as consts,
            tc.tile_pool(name="xpool", bufs=6) as xpool,
            tc.tile_pool(name="eppool", bufs=4) as eppool,
            tc.tile_pool(name="small", bufs=4) as small,
            tc.tile_pool(name="tvp", bufs=8) as tvp,
            tc.tile_pool(name="psacc", bufs=1, space=bass.MemorySpace.PSUM) as psacc,
            tc.tile_pool(name="pstr", bufs=2, space=bass.MemorySpace.PSUM) as pstr,
        ):
            # ---- constants ----
            accs = [psacc.tile([128, 512], f32, tag=f"acc{t}", name=f"acc{t}")
                    for t in range(len(TILES))]
            # PE p-state warm-up: matmul speed ramps 0.65->1.2->2.4 GHz only
            # after ~3us of continuous PE execution. Burn the ramp on dummy
            # 64-col matmuls (junk scratch via a Pool memset at ~60ns, no
            # DMA deps) so every real matmul runs at full clock. The last
            # acc's first real matmul start=True resets the bank.
            junk = consts.tile([128, 64], f32)
            nc.gpsimd.memset(junk, 0.0)
            for wi in range(17):
                nc.tensor.matmul(accs[-1][0:64, 0:64], lhsT=junk, rhs=junk,
                                 start=True, stop=True)
            # w0 hi+lo ride the Pool SWDGE queue, overlapping the SP
            # x-stream pipe start; the SP queue carries ONLY x.
            w0_sb = consts.tile([128, 8, 128], f8e4)
            nc.gpsimd.dma_start(
                out=w0_sb, in_=wt0.rearrange("p (g m) -> p g m", g=8))
            w0l_sb = consts.tile([128, 8, 64], f8e4)
            nc.gpsimd.dma_start(
                out=w0l_sb, in_=wt0l.rearrange("p (g m) -> p g m", g=8))
            bb_sb = consts.tile([128, 1], f32)
            nc.gpsimd.dma_start(out=bb_sb, in_=bb)
            # w1 hi in two pieces + lo in one on ACT so they slot between
            # x groups on the shared transfer pipe
            w1_sb = consts.tile([128, NK - 8, 128], f8e4)
            w1r = wt1.rearrange("p (g m) -> p g m", g=NK - 8)
            nc.scalar.dma_start(out=w1_sb[:, 0:12, :], in_=w1r[:, 0:12, :])
            nc.scalar.dma_start(out=w1_sb[:, 12:24, :], in_=w1r[:, 12:24, :])
            w1l_sb = consts.tile([128, NK - 8, 64], f8e4)
            nc.scalar.dma_start(
                out=w1l_sb, in_=wt1l.rearrange("p (g m) -> p g m", g=NK - 8))
            ident32 = consts.tile([128, 128], f32)
            make_identity(nc, ident32)
            # warm the ACT exp_and_others table during the DMA phase; every
            # later ACT op (Identity/Copy/Exp) stays in this one set.
            warm = consts.tile([1, 1], f32)
            nc.vector.memset(warm, 0.0)
            nc.scalar.add(warm, warm, bb_sb[0:1, 0:1])
            nc.scalar.activation(warm, warm, func=act.Exp)
            # tiles 0-3 stage their results here; one deferred DMA ships
            # them after tile 3's epilogue so no mid-stream output transfer
            # steals a slot on the (now binding) DMA pipe
            final_sb = consts.tile([128, 14], f32)

            for t, bt in enumerate(TILES):
                njs = bt // 128
                off = OFFS[t]
                acc = accs[t][:, 0:bt]
                # ---- stream tile t's contraction, accumulate logits.T ----
                # acc[0:64,:] = WSCALE*noise logits.T, acc[64:128,:] =
                # -WSCALE*expert logits.T (both pre-bias). DoubleRow pairs:
                # one hi + one lo matmul per (2k, 2k+1) chunk pair, all
                # accumulating into one PSUM group.
                base = 128 * NK * off
                xtile = xt[base:base + 128 * NK * bt].rearrange(
                    "(p k b) -> p k b", p=128, k=NK)
                k0 = 0
                for gsz in GROUPS[t]:
                    xk = xpool.tile([128, gsz, bt], f8e4, tag=f"xk{bt}_{gsz}")
                    nc.sync.dma_start(out=xk, in_=xtile[:, k0:k0 + gsz, :])
                    for g in range(0, gsz, 2):
                        k = k0 + g
                        if k < 8:
                            wh = w0_sb[:, k:k + 2, :]
                            wl = w0l_sb[:, k:k + 2, :]
                        else:
                            wh = w1_sb[:, k - 8:k - 6, :]
                            wl = w1l_sb[:, k - 8:k - 6, :]
                        xp = xk[:, g:g + 2, :]
                        nc.tensor.matmul(acc, lhsT=wh, rhs=xp,
                                         start=(k == 0), stop=False,
                                         perf_mode=drow)
                        nc.tensor.matmul(accs[t][0:64, 0:bt], lhsT=wl,
                                         rhs=xp, start=False,
                                         stop=(k == NK - 2),
                                         perf_mode=drow,
                                         skip_group_check=True)
                    k0 += gsz

                # ---- epilogue for tile t (overlaps tile t+1's stream) ----
                # ONE exp for both halves straight off PSUM: the expert
                # weights/bias are host-negated, so e[0:64] = exp(nz+nb) and
                # e[64:128] = exp(-(ez+eb)) share the same +1/WSCALE scale.
                ecomb = eppool.tile([128, bt], f32, tag=f"ec{bt}")
                nc.scalar.activation(ecomb, accs[t][:, 0:bt], func=act.Exp,
                                     scale=1.0 / WSCALE, bias=bb_sb)
                # transpose to batch-major [128 batch, j, 0:64|64:128] in
                # one [128,128] transpose per j-block. Tiles 4/5 stage in
                # the long-retired acc0/acc1 banks so they never wait on
                # the 2-deep pstr rotation.
                if t < 4:
                    ps = pstr.tile([128, 4, 128], f32, tag="ps",
                                   name=f"ps{t}")
                    psC = [ps[:, j, :] for j in range(njs)]
                    psA = [ps[:, j, 0:64] for j in range(njs)]
                    psAall = ps[:, 0:njs, 0:64]
                    psBall = ps[:, 0:njs, 64:128]
                else:
                    psC = [accs[t - 4][:, j * 128:(j + 1) * 128]
                           for j in range(njs)]
                    psA = [accs[t - 4][:, j * 128:j * 128 + 64]
                           for j in range(njs)]
                    psB = [accs[t - 4][:, j * 128 + 64:(j + 1) * 128]
                           for j in range(njs)]
                for j in range(njs):
                    nc.tensor.transpose(psC[j],
                                        ecomb[:, j * 128:(j + 1) * 128],
                                        ident32)
                # den = 1 + eB; sigmoid = 1/den -- emitted before tv/zred so
                # the sig chain (which es and the final select depend on)
                # clears DVE first
                # den/es for the last two tiles run on the idle Pool engine
                # (their staging is in retired acc banks so Pool reading ps
                # cannot stall later transposes, and their outs ride SP so
                # no SWDGE gen competes on Pool); this keeps tile 5's DVE
                # chain from queueing behind tile 4's.
                # den on ACT (identity+bias imm stays in the exp table
                # set): ACT is ~13% busy and this is pure DVE-backlog
                # relief; ACT-den retires long before the DVE stts so the
                # ps release point is unchanged.
                den = small.tile([128, 4, 64], f32, tag="den")
                if t < 4:
                    nc.scalar.add(den[:, 0:njs, :], psBall, 1.0)
                else:
                    for j in range(njs):
                        nc.scalar.add(den[:, j, :], psB[j], 1.0)
                sig = small.tile([128, 4, 64], f32, tag="sig")
                nc.vector.reciprocal(sig[:, 0:njs, :], den[:, 0:njs, :])
                # top-8 on exp(v) (monotone => same selection as on v);
                # zsum = sum of the top-8 values in ONE reduce over tv
                tv = tvp.tile([128, 32], f32, tag="tv", name=f"tv{t}")
                for j in range(njs):
                    nc.vector.max(tv[:, j * 8:(j + 1) * 8], psA[j])
                zsum = small.tile([128, 4], f32, tag="zsum")
                nc.vector.tensor_reduce(
                    zsum[:, 0:njs],
                    tv.rearrange("p (j k) -> p j k", k=8)[:, 0:njs, :],
                    axis=mybir.AxisListType.X, op=alu.add)
                # es = e * sigmoid in ONE strided op (per-j for the last
                # tile, whose staging lives in a plain acc-bank slice). All
                # ps reads stay on DVE so the PSUM staging buffer frees as
                # soon as the DVE chain drains.
                es = small.tile([128, 4, 64], f32, tag="es")
                if t < 4:
                    nc.vector.tensor_mul(es[:, 0:njs, :], psAall,
                                         sig[:, 0:njs, :])
                else:
                    for j in range(njs):
                        nc.vector.tensor_mul(es[:, j, :], psA[j],
                                             sig[:, j, :])
                # s4 = sum of top-8 e*sigmoid (accumulating select)
                s4 = small.tile([128, 4], f32, tag="s4")
                scr = small.tile([128, 4, 64], f32, tag="scr")
                for j in range(njs):
                    nc.vector.scalar_tensor_tensor(
                        out=scr[:, j, :], in0=psA[j],
                        scalar=tv[:, j * 8 + 7:j * 8 + 8], in1=es[:, j, :],
                        op0=alu.is_ge, op1=alu.mult,
                        accum_out=s4[:, j:j + 1])
                rz = small.tile([128, 4], f32, tag="rz")
                nc.vector.reciprocal(rz[:, 0:njs], zsum[:, 0:njs])
                if t <= 3:
                    c0 = off // 128
                    nc.vector.tensor_mul(final_sb[:, c0:c0 + njs],
                                         s4[:, 0:njs], rz[:, 0:njs])
                    if t == 3:
                        nc.gpsimd.dma_start(
                            out=out[0:1792, :].rearrange(
                                "(j p) o -> p (j o)", j=14, p=128),
                            in_=final_sb)
                else:
                    fin = small.tile([128, 4], f32, tag="fin")
                    nc.vector.tensor_mul(fin[:, 0:njs], s4[:, 0:njs],
                                         rz[:, 0:njs])
                    out_t = out[off:off + bt, :].rearrange(
                        "(j p) o -> p (j o)", j=njs, p=128)
                    nc.sync.dma_start(out=out_t, in_=fin[:, 0:njs])

    nc.compile()
    return nc


def get_program():
    if "prog" not in _cached:
        _cached["prog"] = _build_program()
    return _cached["prog"]


def make_in_maps(x, noise_w, noise_b, expert_w, expert_b):
    """Host-side sharding: per-core transposed fp8(e4m3) x + hi/lo weights.

    The expert half is NEGATED (weights and bias) so the kernel computes
    exp(+scale*acc + bias) for all 128 logit rows in one ACT op:
    rows 64:128 then hold exp(-(expert_logit + expert_b)) directly.
    Each weight chunk ships as e4m3 hi + e4m3 residual lo (same x512
    prescale), summed in PSUM by two DoubleRow matmuls.
    """
    import ml_dtypes
    e4 = ml_dtypes.float8_e4m3
    w_comb = np.concatenate([noise_w, -expert_w], axis=0).astype(np.float32)
    wt32 = np.ascontiguousarray(w_comb.T) * np.float32(WSCALE)   # [D, 128]
    # partition p holds [nk, 128] for contraction rows nk*128+p
    wt = np.ascontiguousarray(
        wt32.reshape(NK, 128, 128).transpose(1, 0, 2).reshape(128, NK, 128))
    wh = wt.astype(e4)
    wl = (wt - wh.astype(np.float32)).astype(e4)[:, :, 0:64]  # noise only
    wt0 = np.ascontiguousarray(wh[:, :8, :].reshape(128, -1))
    wt1 = np.ascontiguousarray(wh[:, 8:, :].reshape(128, -1))
    wt0l = np.ascontiguousarray(wl[:, :8, :].reshape(128, -1))
    wt1l = np.ascontiguousarray(wl[:, 8:, :].reshape(128, -1))
    bb = np.concatenate([noise_b, -expert_b]).reshape(128, 1).astype(
        np.float32)
    in_maps = []
    for c in range(NCORES):
        xs = np.ascontiguousarray(x[c * BC:(c + 1) * BC, :].T).astype(e4)
        # per tile: [D, bt] -> [128, NK, bt], concatenated flat
        blocks = []
        for t, bt in enumerate(TILES):
            blk = xs[:, OFFS[t]:OFFS[t] + bt].reshape(NK, 128, bt)
            blocks.append(blk.transpose(1, 0, 2).reshape(-1))
        xr = np.ascontiguousarray(np.concatenate(blocks))
        in_maps.append({"xt": xr, "wt0": wt0, "wt0l": wt0l, "wt1": wt1,
                        "wt1l": wt1l, "bb": bb})
    return in_maps


def kernel(x, noise, router_w, router_b, noise_w, noise_b, expert_w, expert_b,
           _trace=False):
    from concourse.bass_utils import run_bass_kernel_spmd

    x = np.asarray(x, dtype=np.float32)
    nc = get_program()
    in_maps = make_in_maps(x, np.asarray(noise_w), np.asarray(noise_b),
                           np.asarray(expert_w), np.asarray(expert_b))
    res = run_bass_kernel_spmd(nc, in_maps, core_ids=list(range(NCORES)),
                               trace=_trace)
    out = np.concatenate([r["out"] for r in res.results], axis=0)
    if _trace:
        kernel.last_results = res
    return out
